# revision 1
# baseline (speedup 1.0000x reference)
"""Trainium2 Bass kernel for nn_BatchdenseGAT: 2-layer dense GAT, batch 16x512 nodes.

Strategy: pure data-parallel over the batch dim -- 2 graphs per NeuronCore, 8 cores.
Each core runs embedding gather + instance norms + 2 GAT layers + log_softmax for its
2 graphs. No collectives.

Layout choices (per graph, per head):
  - x^T [131, 512] kept feature-major so it feeds matmuls as the moving operand.
  - hp^T [128, 512] (feature-major) from weights-stationary matmuls.
  - attention matrix built directly in TRANSPOSED form E'[m, n] (m=src, n=dst)
    so E' blocks serve as the stationary operand of attn@hp with contraction
    over m on partitions; only adj needs a transpose, once per graph.
  - exp(leaky_relu(s[n]+d[m], 0.2)) = e^{0.2s[n]} * max(e^{0.8s[n]}e^{d[m]}, e^{0.2d[m]})
    where the e^{0.2s[n]} row factor is softmax-row-invariant and dropped. All exps are
    on O(n) vectors; the O(n^2) work is one tensor_scalar + one tensor_tensor per tile.
  - softmax denominators via ones-column matmuls; normalization folded into epilogues.
  - elu(z) = min(exp(z), relu(z)+1) - 1, with the -1 folded into layer 1 as a
    -colsum(w1) bias (computed on device).
"""

import os
import sys
import numpy as np

B, N, V, D_EMB, F0, H = 16, 512, 100000, 64, 64, 8
O1 = O2 = 128
F_IN0 = F0 + D_EMB + 3  # 131
F_IN1 = H * O1          # 1024
EPS = 1e-5
NCORES = 8
G = B // NCORES         # graphs per core = 2
NCH = N // 128          # 4 node chunks

_cache = {}


def _ensure_paths():
    p = "/opt/trn_rl_repo/concourse"
    if os.path.isdir(p) and p not in sys.path:
        sys.path.append(p)


def _build_nc():
    _ensure_paths()
    import concourse.bass as bass
    import concourse.tile as tile
    import concourse.mybir as mybir
    from concourse import bacc
    from concourse.masks import make_identity
    from contextlib import ExitStack

    F32 = mybir.dt.float32
    F32R = mybir.dt.float32r
    BF16 = mybir.dt.bfloat16
    I32 = mybir.dt.int32
    AX = mybir.AxisListType
    OP = mybir.AluOpType
    ACT = mybir.ActivationFunctionType

    nc = bacc.Bacc("TRN2", debug=False, enable_asserts=False)

    d_vert = nc.dram_tensor("vertices", [G, N, 1], I32, kind="ExternalInput").ap()
    d_adj = nc.dram_tensor("adj", [G, N, N], F32, kind="ExternalInput").ap()
    d_h = nc.dram_tensor("hfeat", [G, N, F0], F32, kind="ExternalInput").ap()
    d_ue = nc.dram_tensor("ue", [G, N, 3], F32, kind="ExternalInput").ap()
    d_emb = nc.dram_tensor("emb", [V, D_EMB], F32, kind="ExternalInput").ap()
    d_w0a = nc.dram_tensor("w0a", [H, 128, O1], F32, kind="ExternalInput").ap()
    d_w0b = nc.dram_tensor("w0b", [H, 3, O1], F32, kind="ExternalInput").ap()
    d_w1 = nc.dram_tensor("w1", [H, F_IN1, O2], F32, kind="ExternalInput").ap()
    d_a0 = nc.dram_tensor("a0", [H, O1, 2], F32, kind="ExternalInput").ap()
    d_a1 = nc.dram_tensor("a1", [H, O2, 2], F32, kind="ExternalInput").ap()
    d_n1w = nc.dram_tensor("n1w", [D_EMB, 1], F32, kind="ExternalInput").ap()
    d_n1b = nc.dram_tensor("n1b", [D_EMB, 1], F32, kind="ExternalInput").ap()
    d_n2w = nc.dram_tensor("n2w", [3, 1], F32, kind="ExternalInput").ap()
    d_n2b = nc.dram_tensor("n2b", [3, 1], F32, kind="ExternalInput").ap()
    d_out = nc.dram_tensor("out", [G, N, O2], F32, kind="ExternalOutput").ap()

    with tile.TileContext(nc) as tc, ExitStack() as ctx:
        pers = ctx.enter_context(tc.tile_pool(name="pers", bufs=1))
        wk = ctx.enter_context(tc.tile_pool(name="wk", bufs=2))
        ep_pool = ctx.enter_context(tc.tile_pool(name="ep", bufs=10))
        psm = ctx.enter_context(tc.tile_pool(name="psm", bufs=2, space="PSUM"))
        pst = ctx.enter_context(tc.tile_pool(name="pst", bufs=2, space="PSUM"))
        pss = ctx.enter_context(tc.tile_pool(name="pss", bufs=2, space="PSUM"))
        psd = ctx.enter_context(tc.tile_pool(name="psd", bufs=1, space="PSUM"))

        _cp = [0]

        def anycopy(out, in_):
            # alternate ACT/DVE for psum->sbuf copies to balance engines
            _cp[0] ^= 1
            if _cp[0]:
                nc.scalar.copy(out, in_)
            else:
                nc.vector.tensor_copy(out=out, in_=in_)

        MM = nc.tensor.matmul

        # ---------- persistents ----------
        ident = pers.tile([128, 128], F32, tag="ident")
        make_identity(nc, ident[:])
        ident_b = pers.tile([128, 128], BF16, tag="ident_b")
        make_identity(nc, ident_b[:])

        ones_f = pers.tile([128, 2], F32, tag="ones_f")
        nc.vector.memset(ones_f[:], 1.0)
        ones_b = pers.tile([128, 2], BF16, tag="ones_b")
        nc.vector.memset(ones_b[:], 1.0)
        eights_b = pers.tile([128, 2], BF16, tag="eights_b")
        nc.vector.memset(eights_b[:], float(H))

        # weights -> SBUF, rounded to f32r
        stg = wk.tile([128, H * 128], F32, tag="wstg", bufs=2)
        for h in range(H):
            nc.scalar.dma_start(stg[:, 128 * h:128 * (h + 1)], d_w0a[h])
        w0a_r = pers.tile([128, H * 128], BF16, tag="w0a_r")
        nc.vector.tensor_copy(out=w0a_r[:], in_=stg[:])

        stgb = wk.tile([3, H * 128], F32, tag="wstg", bufs=2, name="stgb")
        for h in range(H):
            nc.scalar.dma_start(stgb[:, 128 * h:128 * (h + 1)], d_w0b[h])
        w0b_r = pers.tile([3, H * 128], BF16, tag="w0b_r")
        nc.vector.tensor_copy(out=w0b_r[:], in_=stgb[:])

        w1_r = []
        for h in range(H):
            s1 = wk.tile([128, F_IN1], F32, tag="wstg", bufs=2, name="s1")
            for k in range(8):
                nc.scalar.dma_start(s1[:, 128 * k:128 * (k + 1)],
                                    d_w1[h, 128 * k:128 * (k + 1), :])
            t1w = pers.tile([128, F_IN1], BF16, tag=f"w1_r{h}")
            nc.vector.tensor_copy(out=t1w[:], in_=s1[:])
            w1_r.append(t1w)

        stga = wk.tile([128, 4 * H], F32, tag="astg")
        for h in range(H):
            nc.sync.dma_start(stga[:, 2 * h:2 * (h + 1)], d_a0[h])
            nc.sync.dma_start(stga[:, 2 * H + 2 * h:2 * H + 2 * (h + 1)], d_a1[h])
        a_r = pers.tile([128, 4 * H], BF16, tag="a_r")
        nc.vector.tensor_copy(out=a_r[:], in_=stga[:])

        def a_pair(layer, h):
            base = 2 * H * layer + 2 * h
            return a_r[:, base:base + 2]

        n1w = pers.tile([D_EMB, 1], F32, tag="n1w")
        n1b = pers.tile([D_EMB, 1], F32, tag="n1b")
        n2w = pers.tile([3, 1], F32, tag="n2w")
        n2b = pers.tile([3, 1], F32, tag="n2b")
        nc.sync.dma_start(n1w[:], d_n1w[:])
        nc.sync.dma_start(n1b[:], d_n1b[:])
        nc.sync.dma_start(n2w[:], d_n2w[:])
        nc.sync.dma_start(n2b[:], d_n2b[:])

        adjT = [[pers.tile([128, N], BF16, tag=f"adjT{g}_{j}", name=f"adjT{g}_{j}") for j in range(NCH)]
                for g in range(G)]
        xTa = [pers.tile([128, N], BF16, tag=f"xTa{g}", name=f"xTa{g}") for g in range(G)]
        xTb = [pers.tile([3, N], BF16, tag=f"xTb{g}", name=f"xTb{g}") for g in range(G)]
        x1T = [[pers.tile([128, N], BF16, tag=f"x1T{g}_{h}", name=f"x1T{g}_{h}") for h in range(H)]
               for g in range(G)]
        acc = [pers.tile([128, N], F32, tag=f"acc{g}", name=f"acc{g}") for g in range(G)]

        # ---------- stage 1: per-graph preprocessing ----------
        def instance_norm_to(dst, src, P, w_col, b_col):
            # dst[P,N] (f32r) = (src - mu) * rstd * w + b, stats over free dim
            s1 = wk.tile([P, 1], F32, tag="in_sum")
            nc.vector.tensor_reduce(s1[:], src[:], AX.X, OP.add)
            sq = wk.tile([P, N], F32, tag="z", name="sq")
            ssq = wk.tile([P, 1], F32, tag="in_ssq")
            nc.scalar.activation(sq[:], src[:], ACT.Square, accum_out=ssq[:])
            mu = wk.tile([P, 1], F32, tag="in_mu")
            nc.vector.tensor_scalar(mu[:], s1[:], 1.0 / N, None, OP.mult)
            ex2 = wk.tile([P, 1], F32, tag="in_ex2")
            nc.vector.tensor_scalar(ex2[:], ssq[:], 1.0 / N, None, OP.mult)
            musq = wk.tile([P, 1], F32, tag="in_musq")
            nc.vector.tensor_tensor(out=musq[:], in0=mu[:], in1=mu[:], op=OP.mult)
            var = wk.tile([P, 1], F32, tag="in_var")
            nc.vector.tensor_tensor(out=var[:], in0=ex2[:], in1=musq[:], op=OP.subtract)
            vare = wk.tile([P, 1], F32, tag="in_vare")
            nc.vector.tensor_scalar(vare[:], var[:], EPS, None, OP.add)
            # rsqrt via quake seed + 3 Newton iterations, all on DVE
            iv = vare[:].bitcast(mybir.dt.int32)
            sh = wk.tile([P, 1], mybir.dt.int32, tag="in_sh")
            nc.vector.tensor_scalar(sh[:], iv, 1, None, OP.arith_shift_right)
            y = wk.tile([P, 1], F32, tag="in_y")
            nc.vector.tensor_scalar(y[:].bitcast(mybir.dt.int32), sh[:], -1,
                                    0x5f3759df, OP.mult, OP.add)
            rstd = y
            for _ in range(3):
                y2 = wk.tile([P, 1], F32, tag="in_y2", name="y2")
                nc.vector.tensor_tensor(out=y2[:], in0=rstd[:], in1=rstd[:], op=OP.mult)
                vy2 = wk.tile([P, 1], F32, tag="in_vy2", name="vy2")
                nc.vector.tensor_tensor(out=vy2[:], in0=vare[:], in1=y2[:], op=OP.mult)
                corr = wk.tile([P, 1], F32, tag="in_corr", name="corr")
                nc.vector.tensor_scalar(corr[:], vy2[:], -0.5, 1.5, OP.mult, OP.add)
                ynew = wk.tile([P, 1], F32, tag="in_ynew", name="ynew")
                nc.vector.tensor_tensor(out=ynew[:], in0=rstd[:], in1=corr[:], op=OP.mult)
                rstd = ynew
            scl = wk.tile([P, 1], F32, tag="in_scl")
            nc.vector.tensor_tensor(out=scl[:], in0=rstd[:], in1=w_col[:], op=OP.mult)
            tb = wk.tile([P, 1], F32, tag="in_tb")
            nc.vector.tensor_tensor(out=tb[:], in0=mu[:], in1=scl[:], op=OP.mult)
            bia = wk.tile([P, 1], F32, tag="in_bia")
            nc.vector.tensor_tensor(out=bia[:], in0=b_col[:], in1=tb[:], op=OP.subtract)
            nc.vector.tensor_scalar(dst, src[:], scl[:], bia[:], OP.mult, OP.add)

        def prep_graph(g):
            # adj transpose: adjT[g][j][m, n-chunk-i] = adj[g][n,m]
            for i in range(NCH):
                ld = wk.tile([128, N], F32, tag="adjld")
                nc.sync.dma_start(ld[:], d_adj[g, 128 * i:128 * (i + 1), :])
                for j in range(NCH):
                    tp = pst.tile([128, 128], F32, tag="tp", bufs=1)
                    nc.tensor.transpose(tp[:], ld[:, 128 * j:128 * (j + 1)], ident[:])
                    anycopy(adjT[g][j][:, 128 * i:128 * (i + 1)], tp[:])

            # h features -> xTa rows 0:64
            for i in range(NCH):
                ldh = wk.tile([128, F0], F32, tag="hld")
                nc.gpsimd.dma_start(ldh[:], d_h[g, 128 * i:128 * (i + 1), :])
                tp = pst.tile([128, 128], F32, tag="tp", bufs=1)
                nc.tensor.transpose(tp[0:F0, :], ldh[:], ident[:])
                anycopy(xTa[g][0:F0, 128 * i:128 * (i + 1)], tp[0:F0, :])

            # embedding gather -> transpose -> instance norm -> xTa rows 64:128
            embT = wk.tile([D_EMB, N], F32, tag="inT", name="embT")
            for i in range(NCH):
                idx = wk.tile([128, 1], I32, tag="idx")
                nc.gpsimd.dma_start(idx[:], d_vert[g, 128 * i:128 * (i + 1), :])
                gat = wk.tile([128, D_EMB], F32, tag="gat")
                nc.gpsimd.indirect_dma_start(
                    out=gat[:], out_offset=None, in_=d_emb[:],
                    in_offset=bass.IndirectOffsetOnAxis(ap=idx[:, :1], axis=0))
                tp = pst.tile([128, 128], F32, tag="tp", bufs=1)
                nc.tensor.transpose(tp[0:D_EMB, :], gat[:], ident[:])
                anycopy(embT[:, 128 * i:128 * (i + 1)], tp[0:D_EMB, :])
            instance_norm_to(xTa[g][F0:F0 + D_EMB, :], embT, D_EMB, n1w, n1b)

            # user_emb -> transpose -> instance norm -> xTb
            ueT = wk.tile([3, N], F32, tag="inT", name="ueT")
            for i in range(NCH):
                ldu = wk.tile([128, 3], F32, tag="ueld")
                nc.gpsimd.dma_start(ldu[:], d_ue[g, 128 * i:128 * (i + 1), :])
                tp = pst.tile([128, 128], F32, tag="tp", bufs=1)
                nc.tensor.transpose(tp[0:3, :], ldu[:], ident[:])
                anycopy(ueT[:, 128 * i:128 * (i + 1)], tp[0:3, :])
            instance_norm_to(xTb[g][:], ueT, 3, n2w, n2b)

        # ---------- attention helper ----------
        def attn_eprime(g, t_r, apr):
            """t_r: tanh(hp^T) [128,N] f32r. Returns 4 E' tiles [m=128, n=N] bf16."""
            s_ps = pss.tile([2, N], F32, tag="sd")
            MM(s_ps[:], apr, t_r[:], start=True, stop=True)  # rows: [s_row; d_row]
            sd_sb = wk.tile([2, N], F32, tag="sdsb")
            nc.scalar.copy(sd_sb[:], s_ps[:])
            # transpose [s;d] row-pairs into column pairs per 128-chunk
            d_ps = psd.tile([128, 2 * NCH], F32, tag="dc")
            for j in range(NCH):
                nc.tensor.transpose(d_ps[:, 2 * j:2 * j + 2],
                                    sd_sb[:, 128 * j:128 * (j + 1)], ident[0:2, 0:2])
            p_row = wk.tile([1, N], BF16, tag="prow")
            nc.scalar.activation(p_row[:], sd_sb[0:1, :], ACT.Exp, scale=0.8)
            p_b = wk.tile([128, N], BF16, tag="pb")
            nc.gpsimd.partition_broadcast(p_b[:], p_row[:])
            acol = wk.tile([128, 2 * NCH], F32, tag="acol")
            nc.scalar.activation(acol[:], d_ps[:], ACT.Exp, scale=0.2)
            qacol = wk.tile([128, 2 * NCH], F32, tag="qacol")
            nc.scalar.activation(qacol[:], d_ps[:], ACT.Exp, scale=1.0)
            eps_ = []
            for j in range(NCH):
                e = ep_pool.tile([128, N], BF16, tag="ep", name="e")
                nc.vector.tensor_scalar(e[:], p_b[:], qacol[:, 2 * j + 1:2 * j + 2],
                                        acol[:, 2 * j + 1:2 * j + 2], OP.mult, OP.max)
                nc.vector.tensor_tensor(out=e[:], in0=e[:], in1=adjT[g][j][:],
                                        op=OP.mult)
                eps_.append(e)
            return eps_

        # ---------- layer 0 (software-pipelined: stageB(i-1) emitted after stageA(i)) ----------
        def l0_stageA(h, g):
            w0a_h = w0a_r[:, 128 * h:128 * (h + 1)]
            w0b_h = w0b_r[:, 128 * h:128 * (h + 1)]
            apr = a_pair(0, h)
            hp_ps = psm.tile([128, N], F32, tag="mm", name="hp_ps")
            MM(hp_ps[:], w0a_h, xTa[g][:], start=True, stop=False)
            MM(hp_ps[:], w0b_h, xTb[g][:], start=False, stop=True)
            t0 = wk.tile([128, N], BF16, tag="tt", name="t0")
            nc.scalar.activation(t0[:], hp_ps[:], ACT.Tanh)
            hp_sb = wk.tile([128, N], BF16, tag="hpsb", name="hp_sb")
            anycopy(hp_sb[:], hp_ps[:])
            hpblk = wk.tile([128, N], BF16, tag="hpblk", name="hpblk")
            for j in range(NCH):
                tp = pst.tile([128, 128], BF16, tag="tpb")
                nc.tensor.transpose(tp[:], hp_sb[:, 128 * j:128 * (j + 1)], ident_b[:])
                anycopy(hpblk[:, 128 * j:128 * (j + 1)], tp[:])
            eps_ = attn_eprime(g, t0, apr)
            return dict(h=h, g=g, hpblk=hpblk, eps=eps_)

        def l0_stageB(st):
            g, h = st["g"], st["h"]
            hpblk, eps_ = st["hpblk"], st["eps"]
            rs_ps = pss.tile([2, N], F32, tag="sd", name="rs_ps")
            for j in range(NCH):
                MM(rs_ps[:], ones_b[:], eps_[j][:],
                   start=(j == 0), stop=(j == NCH - 1))
            rrec = wk.tile([1, N], F32, tag="rrec")
            nc.vector.reciprocal_approx_fast(out=rrec[:], in_=rs_ps[0:1, :])
            rb = wk.tile([128, N], F32, tag="rb")
            nc.gpsimd.partition_broadcast(rb[:], rrec[:])
            out_ps = psm.tile([128, N], F32, tag="mm", name="out_ps")
            for j in range(NCH):
                MM(out_ps[:], hpblk[:, 128 * j:128 * (j + 1)], eps_[j][:],
                   start=(j == 0), stop=(j == NCH - 1))
            z = wk.tile([128, N], F32, tag="z")
            nc.vector.tensor_tensor(out=z[:], in0=out_ps[:], in1=rb[:], op=OP.mult)
            # elu(z)+1 = min(exp(z), relu(z)+1); -1 folded into layer-1 bias
            ez = wk.tile([128, N], F32, tag="ez")
            nc.scalar.activation(ez[:], z[:], ACT.Exp)
            r1 = wk.tile([128, N], F32, tag="r1")
            nc.vector.tensor_scalar(r1[:], z[:], 0.0, 1.0, OP.max, OP.add)
            nc.vector.tensor_tensor(out=x1T[g][h][:], in0=ez[:], in1=r1[:],
                                    op=OP.min)

        prep_graph(0)
        prev = None
        first = True
        for g in range(G):
            for h in range(H):
                cur = l0_stageA(h, g)
                if prev is not None:
                    l0_stageB(prev)
                if first:
                    prep_graph(1)  # overlaps with layer-0 compute on graph 0
                    first = False
                prev = cur
        l0_stageB(prev)

        # -colsum(w1) per head (feeds layer 1 only, so emitted after layer 0)
        cs_ps = pst.tile([128, 2 * H], F32, tag="tp", bufs=1)
        for h in range(H):
            for k in range(8):
                MM(cs_ps[:, 2 * h:2 * h + 2], w1_r[h][:, 128 * k:128 * (k + 1)],
                   ones_b[:], start=(k == 0), stop=(k == 7))
        negcs1 = pers.tile([128, 2 * H], F32, tag="negcs1")
        nc.scalar.mul(negcs1[:], cs_ps[:], -1.0)

        # ---------- layer 1 (pipelined) ----------
        def l1_stageA(h, g):
            apr = a_pair(1, h)
            ncs = negcs1[:, 2 * h:2 * h + 1]
            hp_ps = psm.tile([128, N], F32, tag="mm", name="hp_ps")
            for k in range(8):
                MM(hp_ps[:], w1_r[h][:, 128 * k:128 * (k + 1)], x1T[g][k][:],
                   start=(k == 0), stop=(k == 7))
            t1 = wk.tile([128, N], BF16, tag="tt", name="t1")
            nc.scalar.activation(t1[:], hp_ps[:], ACT.Tanh, bias=ncs)
            hp_sb = wk.tile([128, N], BF16, tag="hpsb", name="hp_sb")
            nc.scalar.activation(hp_sb[:], hp_ps[:], ACT.Identity, bias=ncs)
            hpblk = wk.tile([128, N], BF16, tag="hpblk", name="hpblk")
            for j in range(NCH):
                tp = pst.tile([128, 128], BF16, tag="tpb")
                nc.tensor.transpose(tp[:], hp_sb[:, 128 * j:128 * (j + 1)], ident_b[:])
                anycopy(hpblk[:, 128 * j:128 * (j + 1)], tp[:])
            eps_ = attn_eprime(g, t1, apr)
            return dict(h=h, g=g, hpblk=hpblk, eps=eps_)

        def l1_stageB(st):
            g, h = st["g"], st["h"]
            hpblk, eps_ = st["hpblk"], st["eps"]
            rs_ps = pss.tile([2, N], F32, tag="sd", name="rs_ps")
            for j in range(NCH):
                # eights: folds the mean-over-heads 1/8 into the denominator
                MM(rs_ps[:], eights_b[:], eps_[j][:],
                   start=(j == 0), stop=(j == NCH - 1))
            rrec = wk.tile([1, N], F32, tag="rrec")
            nc.vector.reciprocal_approx_fast(out=rrec[:], in_=rs_ps[0:1, :])
            rb = wk.tile([128, N], F32, tag="rb")
            nc.gpsimd.partition_broadcast(rb[:], rrec[:])
            out_ps = psm.tile([128, N], F32, tag="mm", name="out_ps")
            for j in range(NCH):
                MM(out_ps[:], hpblk[:, 128 * j:128 * (j + 1)], eps_[j][:],
                   start=(j == 0), stop=(j == NCH - 1))
            if h == 0:
                nc.vector.tensor_tensor(out=acc[g][:], in0=out_ps[:], in1=rb[:],
                                        op=OP.mult)
            else:
                contrib = wk.tile([128, N], F32, tag="contrib")
                nc.vector.tensor_tensor(out=contrib[:], in0=out_ps[:], in1=rb[:],
                                        op=OP.mult)
                nc.vector.tensor_tensor(out=acc[g][:], in0=acc[g][:],
                                        in1=contrib[:], op=OP.add)

        prev = None
        for g in range(G):
            for h in range(H):
                cur = l1_stageA(h, g)
                if prev is not None:
                    l1_stageB(prev)
                prev = cur
        l1_stageB(prev)

        # ---------- log_softmax + output (exps batched before lns: one table switch) ----------
        ztps, nmaxs, sexps = [], [], []
        for g in range(G):
            for j in range(NCH):
                ztp = pst.tile([128, 128], F32, tag="tp", bufs=1, name="ztp")
                nc.tensor.transpose(ztp[:], acc[g][:, 128 * j:128 * (j + 1)], ident[:])
                zsb = wk.tile([128, 128], F32, tag="zsb", bufs=8, name="zsb")
                nc.vector.tensor_copy(out=zsb[:], in_=ztp[:])
                nmax = wk.tile([128, 1], F32, tag="nmax", bufs=8, name="nmax")
                nc.vector.tensor_reduce(nmax[:], zsb[:], AX.X, OP.max, negate=True)
                esc = wk.tile([128, 128], F32, tag="esc", name="esc")
                sexp = wk.tile([128, 1], F32, tag="sexp", bufs=8, name="sexp")
                nc.scalar.activation(esc[:], zsb[:], ACT.Exp, bias=nmax[:],
                                     accum_out=sexp[:])
                ztps.append(zsb); nmaxs.append(nmax); sexps.append(sexp)
        fins = []
        for i in range(G * NCH):
            lns = wk.tile([128, 1], F32, tag="lns", bufs=8, name="lns")
            nc.scalar.activation(lns[:], sexps[i][:], ACT.Ln)
            cc = wk.tile([128, 1], F32, tag="cc", bufs=8, name="cc")
            nc.vector.tensor_tensor(out=cc[:], in0=nmaxs[i][:], in1=lns[:],
                                    op=OP.subtract)
            fin = wk.tile([128, 128], F32, tag="fin", bufs=4, name="fin")
            nc.vector.tensor_scalar(fin[:], ztps[i][:], cc[:], None, OP.add)
            fins.append(fin)
        i = 0
        for g in range(G):
            for j in range(NCH):
                nc.sync.dma_start(d_out[g, 128 * j:128 * (j + 1), :], fins[i][:])
                i += 1

    nc.finalize()
    return nc


def _get_nc():
    if "nc" not in _cache:
        _cache["nc"] = _build_nc()
    return _cache["nc"]


def shard_inputs(inputs):
    """Full inputs -> list of 8 per-core input maps."""
    vertices = np.asarray(inputs["vertices"]).astype(np.int32).reshape(B, N, 1)
    adj = np.ascontiguousarray(np.asarray(inputs["adj"], dtype=np.float32))
    h = np.ascontiguousarray(np.asarray(inputs["h"], dtype=np.float32))
    ue = np.ascontiguousarray(np.asarray(inputs["user_emb"], dtype=np.float32))
    emb = np.ascontiguousarray(np.asarray(inputs["emb_table"], dtype=np.float32))
    w0 = np.asarray(inputs["w0"], dtype=np.float32)
    w0a = np.ascontiguousarray(w0[:, :128, :])
    w0b = np.ascontiguousarray(w0[:, 128:, :])
    w1 = np.ascontiguousarray(np.asarray(inputs["w1"], dtype=np.float32))
    a0 = np.ascontiguousarray(np.stack(
        [np.asarray(inputs["a_src0"])[..., 0], np.asarray(inputs["a_dst0"])[..., 0]],
        axis=-1).astype(np.float32))
    a1 = np.ascontiguousarray(np.stack(
        [np.asarray(inputs["a_src1"])[..., 0], np.asarray(inputs["a_dst1"])[..., 0]],
        axis=-1).astype(np.float32))
    n1w = np.asarray(inputs["norm1_w"], dtype=np.float32).reshape(D_EMB, 1)
    n1b = np.asarray(inputs["norm1_b"], dtype=np.float32).reshape(D_EMB, 1)
    n2w = np.asarray(inputs["norm2_w"], dtype=np.float32).reshape(3, 1)
    n2b = np.asarray(inputs["norm2_b"], dtype=np.float32).reshape(3, 1)

    maps = []
    for c in range(NCORES):
        sl = slice(G * c, G * (c + 1))
        maps.append({
            "vertices": np.ascontiguousarray(vertices[sl]),
            "adj": adj[sl], "hfeat": h[sl], "ue": ue[sl], "emb": emb,
            "w0a": w0a, "w0b": w0b, "w1": w1, "a0": a0, "a1": a1,
            "n1w": n1w, "n1b": n1b, "n2w": n2w, "n2b": n2b,
        })
    return maps


def kernel(**inputs):
    _ensure_paths()
    from concourse import bass_utils
    nc = _get_nc()
    maps = shard_inputs(inputs)
    res = bass_utils.run_bass_kernel_spmd(nc, maps, core_ids=list(range(NCORES)))
    out = np.concatenate([res.results[c]["out"] for c in range(NCORES)], axis=0)
    return out

